# revision 6
# baseline (speedup 1.0000x reference)
"""BertSum attention kernel for 8 Trainium2 NeuronCores.

Problem: B=4, S=2048, D=1024, H=16 heads (DH=64).
  q = (data @ Wq.T + bq)/8 ; k = data @ Wk.T + bk ; v = data @ Wv.T + bv
  scores = q k^T, masked (mask=True -> -1e18), softmax, ctx = attn v
  out = ctx @ Wo.T + bo

Sharding: (batch, query-half) -> 8 cores. Each core handles 1024 query
rows of one batch; K/V are computed over the full 2048 keys of that
batch. No collectives; per-core outputs concatenate to the full output.

Per-core layout strategy (all transposed-scores, no PE transposes):
  - dataT [D, S] with the core's own 1024 query rows permuted first.
  - qT/kT computed channel-major ([ch, seq]); v computed [seq, ch] and
    augmented with a ones column per head (gives softmax denominator as
    row 64 of the ctx matmul's PSUM output for free).
  - scoresT [s, q] = kT_h^T-slices x qT_h  (K=64 matmuls, fp32r)
  - exp on ACT (no max-subtraction needed: |scores| is small, masked
    entries are handled by multiplying exp by a 0/1 bf16 mask).
  - ctx^T [65, q] accumulates v_aug^T x expT in PSUM (bf16 matmuls).
  - per-head normalization: reciprocal of denom row, broadcast across
    partitions with a K=1 ones matmul, multiply on DVE.
  - out = ctx_n @ Wo.T + (bo + Wo @ bv), bias added via K=1 ones matmul.
"""

import numpy as np
from contextlib import ExitStack

import ml_dtypes

import concourse.bass as bass
import concourse.mybir as mybir
from concourse import bacc
from concourse.tile import TileContext
from concourse.bass_utils import run_bass_kernel_spmd

F32 = mybir.dt.float32
F32R = mybir.dt.float32r
BF16 = mybir.dt.bfloat16
AF = mybir.ActivationFunctionType
ALU = mybir.AluOpType

B, S, D = 4, 2048, 1024
H, DH = 16, 64
SQ = 1024  # query rows per core
NP = 8  # head pairs (128 channels each)

_CACHE = {}


def _build():
    nc = bacc.Bacc("TRN2", target_bir_lowering=False)

    dataT = nc.declare_dram_parameter("dataT", [D, S], F32R, isOutput=False)
    maskT = nc.declare_dram_parameter("maskT", [S, SQ], BF16, isOutput=False)
    wqT = nc.declare_dram_parameter("wqT", [D, D], F32R, isOutput=False)
    wkT = nc.declare_dram_parameter("wkT", [D, D], F32R, isOutput=False)
    wvT = nc.declare_dram_parameter("wvT", [D, D], F32R, isOutput=False)
    woT = nc.declare_dram_parameter("woT", [D, D], BF16, isOutput=False)
    bq2 = nc.declare_dram_parameter("bq2", [128, NP], F32, isOutput=False)
    bk2 = nc.declare_dram_parameter("bk2", [128, NP], F32, isOutput=False)
    boe = nc.declare_dram_parameter("boe", [1, D], BF16, isOutput=False)
    ones_r = nc.declare_dram_parameter("ones_r", [1, 128], F32R,
                                       isOutput=False)
    ones_b = nc.declare_dram_parameter("ones_b", [1, 128], BF16,
                                       isOutput=False)
    out = nc.declare_dram_parameter("out", [SQ, D], F32, isOutput=True)

    kspill = nc.dram_tensor("kspill", [NP, 128, S], F32R)
    vspill = nc.dram_tensor("vspill", [S // 128, 128, H * 65], BF16)

    with ExitStack() as ctx:
        ctx.enter_context(nc.allow_low_precision(
            reason="fp32r matmul operand prep; accumulation stays f32"))
        tc = ctx.enter_context(TileContext(nc))
        const = ctx.enter_context(tc.tile_pool(name="const", bufs=1))
        wpool = ctx.enter_context(tc.tile_pool(name="w", bufs=1))
        ctxp = ctx.enter_context(tc.tile_pool(name="ctxT", bufs=1))

        ones = const.tile([1, 128], F32R)
        nc.sync.dma_start(out=ones, in_=ones_r[:, :])
        onesb = const.tile([1, 128], BF16)
        nc.sync.dma_start(out=onesb, in_=ones_b[:, :])
        boesb = const.tile([1, D], BF16)
        nc.sync.dma_start(out=boesb, in_=boe[:, :])
        bqsb = const.tile([128, NP], F32)
        nc.sync.dma_start(out=bqsb, in_=bq2[:, :])
        bksb = const.tile([128, NP], F32)
        nc.sync.dma_start(out=bksb, in_=bk2[:, :])

        def load_w(w, dt_w=F32R):
            ts = []
            for i in range(8):
                t = wpool.tile([128, D], dt_w, tag=f"w{i}", name=f"wsb{i}")
                nc.sync.dma_start(out=t, in_=w[i * 128:(i + 1) * 128, :])
                ts.append(t)
            return ts

        def load_data_chunk(dpool, sc):
            dsb = dpool.tile([128, 8, 512], F32R, tag="d")
            for i in range(8):
                nc.sync.dma_start(
                    out=dsb[:, i, :],
                    in_=dataT[i * 128:(i + 1) * 128, sc * 512:(sc + 1) * 512],
                )
            return dsb

        qpool = ctx.enter_context(tc.tile_pool(name="qT", bufs=1))
        qT = [qpool.tile([128, SQ], F32R, tag=f"q{p}", name=f"qT{p}")
              for p in range(NP)]

        # ---------------- Phase P1: kT projection, spilled to DRAM -------
        with ExitStack() as pctx:
            dpool = pctx.enter_context(tc.tile_pool(name="dstream", bufs=2))
            spool = pctx.enter_context(tc.tile_pool(name="pstage", bufs=3))
            psp = pctx.enter_context(
                tc.tile_pool(name="psproj", bufs=4, space="PSUM"))

            wsb = load_w(wkT)
            for sc in range(4):
                dsb = load_data_chunk(dpool, sc)
                for p in range(NP):
                    ps = psp.tile([128, 512], F32, tag="ps")
                    for i in range(8):
                        nc.tensor.matmul(
                            ps,
                            wsb[i][:, p * 128:(p + 1) * 128],
                            dsb[:, i, :],
                            start=(i == 0), stop=(i == 7),
                        )
                    st = spool.tile([128, 512], F32R, tag="kst")
                    nc.vector.tensor_scalar_add(st, ps, bksb[:, p:p + 1])
                    nc.sync.dma_start(
                        out=kspill[p, :, sc * 512:(sc + 1) * 512], in_=st)

            # ------------- Phase P2: v projection (augmented, bf16) ------
            wsb = load_w(wvT)
            for sc in range(4):
                dsb = load_data_chunk(dpool, sc)
                for ss in range(4):
                    vstage = spool.tile([128, H * 65], BF16, tag="vst")
                    for half in range(2):
                        ps = psp.tile([128, 512], F32, tag="ps")
                        for i in range(8):
                            nc.tensor.matmul(
                                ps,
                                dsb[:, i, ss * 128:(ss + 1) * 128],
                                wsb[i][:, half * 512:(half + 1) * 512],
                                start=(i == 0), stop=(i == 7),
                            )
                        dstv = vstage[:, half * 520:(half + 1) * 520]
                        dstv = dstv.rearrange("p (h c) -> p h c", c=65)
                        nc.vector.tensor_copy(
                            out=dstv[:, :, 0:64],
                            in_=ps.rearrange("p (h c) -> p h c", c=64),
                        )
                    onescol = vstage.rearrange("p (h c) -> p h c", c=65)
                    nc.vector.memset(onescol[:, :, 64:65], 1.0)
                    nc.sync.dma_start(out=vspill[sc * 4 + ss, :, :], in_=vstage)

            # ------------- Phase P3: qT projection (resident) ------------
            wsb = load_w(wqT)
            for sc in range(2):
                dsb = load_data_chunk(dpool, sc)
                for p in range(NP):
                    ps = psp.tile([128, 512], F32, tag="ps")
                    for i in range(8):
                        nc.tensor.matmul(
                            ps,
                            wsb[i][:, p * 128:(p + 1) * 128],
                            dsb[:, i, :],
                            start=(i == 0), stop=(i == 7),
                        )
                    nc.vector.tensor_scalar(
                        out=qT[p][:, sc * 512:(sc + 1) * 512],
                        in0=ps,
                        scalar1=0.125,
                        scalar2=bqsb[:, p:p + 1],
                        op0=ALU.mult,
                        op1=ALU.add,
                    )

        # ---------------- Phase A: attention -----------------------------
        with ExitStack() as actx:
            mpool = actx.enter_context(tc.tile_pool(name="mask", bufs=1))
            msb = []
            for i in range(S // 128):
                t = mpool.tile([128, SQ], BF16, tag=f"m{i}")
                nc.sync.dma_start(out=t, in_=maskT[i * 128:(i + 1) * 128, :])
                msb.append(t)

            kpool = actx.enter_context(tc.tile_pool(name="kp", bufs=2))
            vpool = actx.enter_context(tc.tile_pool(name="vp", bufs=2))
            epool = actx.enter_context(tc.tile_pool(name="exp", bufs=3))
            rpool = actx.enter_context(tc.tile_pool(name="rec", bufs=2))
            cspool = actx.enter_context(tc.tile_pool(name="cstage", bufs=2))
            pss = actx.enter_context(
                tc.tile_pool(name="pss", bufs=2, space="PSUM"))
            psc = actx.enter_context(
                tc.tile_pool(name="psc", bufs=2, space="PSUM"))

            ctxT = [ctxp.tile([128, SQ], BF16, tag=f"ctx{p}", name=f"ctxT{p}")
                    for p in range(NP)]

            for p in range(NP):
                ksb = kpool.tile([128, S], F32R, tag="k")
                nc.sync.dma_start(out=ksb, in_=kspill[p, :, :])
                vsb = vpool.tile([128, 16, 130], BF16, tag="v")
                for st in range(16):
                    nc.sync.dma_start(
                        out=vsb[:, st, :],
                        in_=vspill[st, :, p * 130:(p + 1) * 130])

                for h in range(2):
                    cps = psc.tile([128, SQ], F32, tag="ctxps")
                    for i in range(16):
                        ss = pss.tile([128, SQ], F32, tag="sc")
                        for qh in range(2):
                            nc.tensor.matmul(
                                ss[:, qh * 512:(qh + 1) * 512],
                                ksb[h * 64:(h + 1) * 64,
                                    i * 128:(i + 1) * 128],
                                qT[p][h * 64:(h + 1) * 64,
                                       qh * 512:(qh + 1) * 512],
                                start=True, stop=True,
                            )
                        et = epool.tile([128, SQ], BF16, tag="e")
                        nc.scalar.activation(out=et, in_=ss, func=AF.Exp)
                        em = epool.tile([128, SQ], BF16, tag="em")
                        nc.vector.tensor_mul(em, et, msb[i])
                        for qh in range(2):
                            nc.tensor.matmul(
                                cps[0:65, qh * 512:(qh + 1) * 512],
                                vsb[:, i, h * 65:(h + 1) * 65],
                                em[:, qh * 512:(qh + 1) * 512],
                                start=(i == 0), stop=(i == 15),
                            )
                    rec32 = rpool.tile([1, SQ], F32, tag="r32")
                    nc.vector.reciprocal(rec32, cps[64:65, :])
                    rec = rpool.tile([1, SQ], F32R, tag="r")
                    nc.vector.tensor_scalar_mul(rec, rec32, 1.0)
                    bc = pss.tile([128, SQ], F32, tag="sc")
                    for qh in range(2):
                        nc.tensor.matmul(
                            bc[0:64, qh * 512:(qh + 1) * 512],
                            ones[0:1, 0:64],
                            rec[0:1, qh * 512:(qh + 1) * 512],
                            start=True, stop=True,
                        )
                    cs = cspool.tile([64, SQ], F32, tag="cs")
                    nc.vector.tensor_copy(cs, cps[0:64, :])
                    nc.vector.tensor_mul(
                        ctxT[p][h * 64:(h + 1) * 64, :], cs, bc[0:64, :])

        # ---------------- Phase O: output projection ----------------------
        with ExitStack() as octx:
            wsb = load_w(woT, BF16)
            opool = octx.enter_context(tc.tile_pool(name="ost", bufs=3))
            pso = octx.enter_context(
                tc.tile_pool(name="pso", bufs=4, space="PSUM"))
            for qt in range(8):
                for dh in range(2):
                    ps = pso.tile([128, 512], F32, tag="o")
                    for p in range(NP):
                        nc.tensor.matmul(
                            ps,
                            ctxT[p][:, qt * 128:(qt + 1) * 128],
                            wsb[p][:, dh * 512:(dh + 1) * 512],
                            start=(p == 0), stop=False,
                        )
                    nc.tensor.matmul(
                        ps,
                        onesb[0:1, 0:128],
                        boesb[0:1, dh * 512:(dh + 1) * 512],
                        start=False, stop=True,
                    )
                    ot = opool.tile([128, 512], F32, tag="ot")
                    nc.vector.tensor_copy(ot, ps)
                    nc.sync.dma_start(
                        out=out[qt * 128:(qt + 1) * 128,
                                dh * 512:(dh + 1) * 512],
                        in_=ot)

    nc.finalize()
    return nc


def _get_nc():
    if "nc" not in _CACHE:
        _CACHE["nc"] = _build()
    return _CACHE["nc"]


def _prep_inputs(data, mask, Wq, bq, Wk, bk, Wv, bv, Wo, bo):
    data = np.asarray(data, dtype=np.float32)
    mask = np.asarray(mask)
    WqT = np.ascontiguousarray(np.asarray(Wq, np.float32).T)
    WkT = np.ascontiguousarray(np.asarray(Wk, np.float32).T)
    WvT = np.ascontiguousarray(np.asarray(Wv, np.float32).T)
    WoT = np.ascontiguousarray(np.asarray(Wo, np.float32).T
                               .astype(ml_dtypes.bfloat16))
    bq2 = np.ascontiguousarray((np.asarray(bq, np.float32) / 8.0)
                               .reshape(NP, 128).T)
    bk2 = np.ascontiguousarray(np.asarray(bk, np.float32)
                               .reshape(NP, 128).T)
    boe = (np.asarray(bo, np.float32)
           + np.asarray(Wo, np.float32) @ np.asarray(bv, np.float32))
    boe = np.ascontiguousarray(boe.reshape(1, D)).astype(ml_dtypes.bfloat16)
    ones_r = np.ones((1, 128), np.float32)
    ones_b = np.ones((1, 128), ml_dtypes.bfloat16)

    in_maps = []
    for c in range(8):
        b, half = divmod(c, 2)
        q0 = half * SQ
        perm = np.concatenate(
            [np.arange(q0, q0 + SQ), np.arange((1 - half) * SQ,
                                               (1 - half) * SQ + SQ)])
        dT = np.ascontiguousarray(data[b].T[:, perm])
        keep = ~mask[b, q0:q0 + SQ, :]  # [SQ, S] True where attended
        mT = np.ascontiguousarray(
            keep.T[perm, :].astype(ml_dtypes.bfloat16))
        in_maps.append({
            "dataT": dT, "maskT": mT,
            "wqT": WqT, "wkT": WkT, "wvT": WvT, "woT": WoT,
            "bq2": bq2, "bk2": bk2, "boe": boe,
            "ones_r": ones_r, "ones_b": ones_b,
        })
    return in_maps


def kernel(**inputs):
    in_maps = _prep_inputs(**inputs)
    nc = _get_nc()
    res = run_bass_kernel_spmd(nc, in_maps, list(range(8))).results
    out = np.empty((B, S, D), np.float32)
    for c in range(8):
        b, half = divmod(c, 2)
        out[b, half * SQ:(half + 1) * SQ, :] = res[c]["out"]
    return out
